# revision 3
# baseline (speedup 1.0000x reference)
"""Trainium2 Bass kernel for CombinedSARAFilter (bf16 I/O blocked linear scan).

Math: with D_t = I_t - I_{t-1} (I_{-1}=0), the module reduces to
    x_t = lam_r x_{t-1} + p D_t + q I_t
    o_t = lam_d o_{t-1} + a_d x_t + c3 |D_t|        (out = o, since TAU_RA == TAU_D)
Device computes in ramp-shifted space o~_t = o_t - g_t, g_t = MU(1-lam_d^{t+1}),
which turns the mean ramp into a constant per-step forcing -a_d*MU (applied as a
per-partition bias on the PSUM->SBUF copy) and keeps device values near zero so
bf16 I/O is accurate. Host adds g_t back and upcasts.

Blocked linear scan: time chunks of L=125 on SBUF partitions; one [128, 2048]
bf16 rhs tile per chunk holds 2 carry rows (x, o~) + 126 input rows; per
512-lane block three bf16 matmuls on TensorE (first-difference WD, combined
linear response W1 incl. state propagation, abs response WA) accumulate in
PSUM; ScalarE does c3*|D|; VectorE applies the bias and casts to bf16; carries
ride along in the out tile and hop to the next rhs tile via SBUF->SBUF DMA.
"""
import sys

sys.path.insert(0, "/opt/trn_rl_repo")

import numpy as np
import ml_dtypes

BF16 = ml_dtypes.bfloat16

# filter constants
DT = 0.1
TAU_RA, K3 = 30.0, 2.0
TAU_R, TAU_D, K1, K2 = 5.0, 30.0, 0.05, 3.0
A_R = DT / TAU_R
A_D = DT / TAU_D
LAM_R = 1.0 - A_R
LAM_D = 1.0 - A_D
P = A_R * K2 / DT
Q = A_R * K1
C3 = K3 / TAU_RA
MU = 22.5  # approx steady-state E[out]; only affects rounding error

B, T, N = 8, 2000, 2048
L = 125            # time chunk (on partitions)
NCH = T // L       # 16
NB = 512           # PSUM bank = 512 fp32 lanes
NH = 1024          # half width (2 banks)


def build_weights():
    """Host-side fp64 construction of the chunk filter matrices (bf16 on device)."""
    i = np.arange(L)
    Mr = np.tril(LAM_R ** np.clip(i[:, None] - i[None, :], 0, None))
    Md = np.tril(LAM_D ** np.clip(i[:, None] - i[None, :], 0, None))
    Bp = np.zeros((L, L + 1))
    Bp[i, i + 1] = 1.0
    Bp[i, i] = -1.0
    U = P * Bp
    U[:, 1:] += Q * np.eye(L)
    Rx = Mr @ U                           # x response to ihat [125,126]
    F1 = A_D * Md @ Rx                    # o~ response to ihat
    v1 = LAM_D ** (i + 1)                 # o~ response to o~_in
    v2 = A_D * (Md @ (LAM_R ** (i + 1)))  # o~ response to x_in
    ones_resp = -A_D * MU * Md.sum(1)     # per-row response to -a_d*MU forcing

    # W1: [127 out rows, 128 in rows]; in rows: 0=x_in, 1=o~_in, 2+j=ihat_j
    # out rows: 0=x_out, 1=o~_out, 2+i=o~_i
    W1 = np.zeros((127, 128))
    W1[0, 0] = LAM_R ** L
    W1[0, 2:] = Rx[L - 1]
    W1[1, 0] = v2[L - 1]
    W1[1, 1] = LAM_D ** L
    W1[1, 2:] = F1[L - 1]
    W1[2:, 0] = v2
    W1[2:, 1] = v1
    W1[2:, 2:] = F1

    # WD: [125 out, 128 in]: D_i = rhs[3+i] - rhs[2+i]
    WD = np.zeros((125, 128))
    WD[i, 3 + i] = 1.0
    WD[i, 2 + i] = -1.0

    # WA: [127 out, 125 in]: response to a = c3|D| (enters o directly via I_RA)
    WA = np.zeros((127, 125))
    WA[1] = Md[L - 1]
    WA[2:] = Md

    bias = np.zeros((127, 1))
    bias[1, 0] = ones_resp[L - 1]
    bias[2:, 0] = ones_resp[:, None][:, 0]

    return {
        "W1_T": np.ascontiguousarray(W1.T).astype(BF16),   # [128, 127]
        "WD_T": np.ascontiguousarray(WD.T).astype(BF16),   # [128, 125]
        "WA_T": np.ascontiguousarray(WA.T).astype(BF16),   # [125, 127]
        "BIAS": bias.astype(np.float32),                   # [127, 1]
        "S_INIT": np.zeros((3, N), BF16),                  # x=0, o~=0, I_{-1}=0
    }


def build_program(reps: int = 1):
    """Emit the single-core SPMD program. Returns (nc, weight_arrays)."""
    from concourse import bacc, mybir, tile

    dt = mybir.dt
    w = build_weights()
    wdtypes = {
        "W1_T": dt.bfloat16, "WD_T": dt.bfloat16, "WA_T": dt.bfloat16,
        "BIAS": dt.float32, "S_INIT": dt.bfloat16,
    }

    nc = bacc.Bacc("TRN2", target_bir_lowering=False, debug=False)

    X = nc.dram_tensor("X", [T, N], dt.bfloat16, kind="ExternalInput")
    Y = nc.dram_tensor("Y", [T, N], dt.bfloat16, kind="ExternalOutput")
    wd = {
        name: nc.dram_tensor(name, list(arr.shape), wdtypes[name], kind="ExternalInput")
        for name, arr in w.items()
    }

    with tile.TileContext(nc) as tc:
        with (
            tc.tile_pool(name="wpool", bufs=1) as wpool,
            tc.tile_pool(name="io", bufs=3) as io,
            tc.tile_pool(name="opool", bufs=3) as opool,
            tc.tile_pool(name="apool", bufs=3) as apool,
            tc.tile_pool(name="psO", bufs=2, space="PSUM") as psO,
            tc.tile_pool(name="psD", bufs=2, space="PSUM") as psD,
        ):
            # weights -> SBUF once
            wt = {}
            for name, arr in w.items():
                t_ = wpool.tile(list(arr.shape), wdtypes[name], tag=name)
                nc.sync.dma_start(out=t_[:], in_=wd[name][:])
                wt[name] = t_

            for rep in range(reps):
                # first chunk's rhs: states+I_{-1} from S_INIT, inputs from X
                rhs = io.tile([128, N], dt.bfloat16, tag="rhs")
                nc.sync.dma_start(out=rhs[0:3, :], in_=wd["S_INIT"][:])
                nc.sync.dma_start(out=rhs[3:128, :], in_=X[0:L, :])

                for k in range(NCH):
                    if k + 1 < NCH:
                        rhs_next = io.tile([128, N], dt.bfloat16, tag="rhs")
                        nc.sync.dma_start(
                            out=rhs_next[2:128, :],
                            in_=X[(k + 1) * L - 1:(k + 2) * L, :],
                        )
                    out_t = opool.tile([127, N], dt.bfloat16, tag="out")

                    ps_d = [psD.tile([L, NH], dt.float32, tag="D", name=f"psd{h}") for h in range(2)]
                    for h in range(2):
                        for b in range(2):
                            c0 = b * NB
                            nc.tensor.matmul(
                                ps_d[h][:, c0:c0 + NB],
                                wt["WD_T"][:],
                                rhs[:, h * NH + c0:h * NH + c0 + NB],
                                start=True, stop=True,
                            )
                    a_t = [apool.tile([L, NH], dt.bfloat16, tag="A", name=f"a{h}") for h in range(2)]
                    for h in range(2):
                        nc.scalar.activation(
                            a_t[h][:], ps_d[h][:],
                            func=mybir.ActivationFunctionType.Abs,
                            scale=float(C3),
                        )

                    ps_o = [psO.tile([127, NH], dt.float32, tag="O", name=f"pso{h}") for h in range(2)]
                    for h in range(2):
                        for b in range(2):
                            c0 = b * NB
                            nc.tensor.matmul(
                                ps_o[h][:, c0:c0 + NB],
                                wt["W1_T"][:],
                                rhs[:, h * NH + c0:h * NH + c0 + NB],
                                start=True, stop=False,
                            )
                    for h in range(2):
                        for b in range(2):
                            c0 = b * NB
                            nc.tensor.matmul(
                                ps_o[h][:, c0:c0 + NB],
                                wt["WA_T"][:],
                                a_t[h][:, c0:c0 + NB],
                                start=False, stop=True,
                            )

                    # bias add + fp32->bf16 cast (carries ride in rows 0:2)
                    for h in range(2):
                        nc.vector.tensor_scalar(
                            out=out_t[:, h * NH:(h + 1) * NH],
                            in0=ps_o[h][:],
                            scalar1=wt["BIAS"][:],
                            scalar2=None,
                            op0=mybir.AluOpType.add,
                        )

                    # carry hop to next chunk's rhs (SBUF->SBUF DMA, 2 rows)
                    if k + 1 < NCH:
                        nc.sync.dma_start(
                            out=rhs_next[0:2, :], in_=out_t[0:2, :]
                        )
                    nc.sync.dma_start(
                        out=Y[k * L:(k + 1) * L, :], in_=out_t[2:127, :]
                    )
                    rhs = rhs_next if k + 1 < NCH else None

    nc.compile()
    return nc, w


_PROGRAM_CACHE = {}


def _get_program():
    if "nc" not in _PROGRAM_CACHE:
        nc, w = build_program()
        _PROGRAM_CACHE["nc"] = nc
        _PROGRAM_CACHE["w"] = w
    return _PROGRAM_CACHE["nc"], _PROGRAM_CACHE["w"]


def kernel(I_in: np.ndarray) -> np.ndarray:
    """Full-input entry point: I_in [8, 2000, 2048] fp32 -> out same shape."""
    from concourse.bass_utils import run_bass_kernel_spmd

    nc, w = _get_program()
    I_bf = np.ascontiguousarray(I_in, dtype=np.float32).astype(BF16)
    in_maps = [
        {"X": I_bf[b], **{name: arr for name, arr in w.items()}}
        for b in range(B)
    ]
    last_err = None
    for _attempt in range(3):
        try:
            res = run_bass_kernel_spmd(nc, in_maps, list(range(B)))
            out = np.stack(
                [np.asarray(res.results[b]["Y"]).astype(np.float32) for b in range(B)],
                axis=0,
            )
            g = (MU * (1.0 - LAM_D ** (np.arange(T, dtype=np.float64) + 1))).astype(
                np.float32
            )
            return out + g[None, :, None]
        except Exception as e:  # transient device errors: retry
            last_err = e
            import time as _time
            _time.sleep(5)
    raise last_err


if __name__ == "__main__":
    rng = np.random.default_rng(0)
    I = rng.standard_normal((B, T, N), dtype=np.float32)
    out = kernel(I)
    print(out.shape, out.dtype, np.abs(out).max())


# revision 11
# speedup vs baseline: 3.1100x; 3.1100x over previous
"""Trainium2 Bass kernel for CombinedSARAFilter (bf16 I/O blocked linear scan).

Math: with D_t = I_t - I_{t-1} (I_{-1}=0), the module reduces to
    x_t = lam_r x_{t-1} + p D_t + q I_t
    o_t = lam_d o_{t-1} + a_d x_t + c3 |D_t|        (out = o, since TAU_RA == TAU_D)
Device computes in ramp-shifted space o~_t = o_t - g_t, g_t = MU(1-lam_d^{t+1}),
which turns the mean ramp into a constant per-step forcing -a_d*MU (applied as a
per-partition bias on the PSUM->SBUF copy) and keeps device values near zero so
bf16 I/O stays accurate. Host adds g_t back and upcasts.

Blocked linear scan, time chunks of L=125 on SBUF partitions. Per chunk:
  - one [126, 2048] bf16 rhs tile (input rows I_{kL-1..kL+124}), one bf16 DMA
  - PE: per 512-lane bank, WD (first difference, to PSUM), W1N (input
    response), WA (abs response), and a K=2 WS matmul that reads the carry
    rows DIRECTLY from the previous chunk's out tile (rows 0-1) -- no state
    extraction ops on the serial path (compute engines cannot address
    partition base 1, so the diff must be a PE matmul)
  - ACT: a = c3|D| from PSUM, per half
  - DVE: per-half PSUM->SBUF copy with per-partition bias, cast to bf16;
    carry rows ride along in rows 0-1 of the out tile
  - bf16 out DMA per half
"""
import sys

sys.path.insert(0, "/opt/trn_rl_repo")

import numpy as np
import ml_dtypes

BF16 = ml_dtypes.bfloat16

# filter constants
DT = 0.1
TAU_RA, K3 = 30.0, 2.0
TAU_R, TAU_D, K1, K2 = 5.0, 30.0, 0.05, 3.0
A_R = DT / TAU_R
A_D = DT / TAU_D
LAM_R = 1.0 - A_R
LAM_D = 1.0 - A_D
P = A_R * K2 / DT
Q = A_R * K1
C3 = K3 / TAU_RA
MU = 22.5  # approx steady-state E[out]; only affects rounding error

B, T, N = 8, 2000, 2048
L = 125            # time chunk (on partitions)
NCH = T // L       # 16
NB = 512           # PSUM bank = 512 fp32 lanes
NH = 1024          # half width (2 banks)


def build_weights():
    """Host-side fp64 construction of the chunk filter matrices (bf16 on device)."""
    i = np.arange(L)
    Mr = np.tril(LAM_R ** np.clip(i[:, None] - i[None, :], 0, None))
    Md = np.tril(LAM_D ** np.clip(i[:, None] - i[None, :], 0, None))
    Bp = np.zeros((L, L + 1))
    Bp[i, i + 1] = 1.0
    Bp[i, i] = -1.0
    U = P * Bp
    U[:, 1:] += Q * np.eye(L)
    Rx = Mr @ U                           # x response to ihat [125,126]
    F1 = A_D * Md @ Rx                    # o~ response to ihat
    v1 = LAM_D ** (i + 1)                 # o~ response to o~_in
    v2 = A_D * (Md @ (LAM_R ** (i + 1)))  # o~ response to x_in
    ones_resp = -A_D * MU * Md.sum(1)     # per-row response to -a_d*MU forcing

    # out rows: 0=x_out, 1=o~_out, 2+i=o~_i
    # W1N: [127 out, 126 in]: response to ihat rows j=0..125 (I_{kL-1+j})
    W1N = np.zeros((127, 126))
    W1N[0] = Rx[L - 1]
    W1N[1] = F1[L - 1]
    W1N[2:] = F1

    # WS: [127 out, 2 in]: response to carries (x_in, o~_in)
    WS = np.zeros((127, 2))
    WS[0, 0] = LAM_R ** L
    WS[1, 0] = v2[L - 1]
    WS[1, 1] = LAM_D ** L
    WS[2:, 0] = v2
    WS[2:, 1] = v1

    # WA: [127 out, 125 in]: response to a = c3|D| (enters o directly via I_RA)
    WA = np.zeros((127, 125))
    WA[1] = Md[L - 1]
    WA[2:] = Md

    # WD: [125 out, 126 in]: D_i = rhs[1+i] - rhs[i]
    WDm = np.zeros((125, 126))
    WDm[i, 1 + i] = 1.0
    WDm[i, i] = -1.0

    bias = np.zeros((127, 1))
    bias[1, 0] = ones_resp[L - 1]
    bias[2:, 0] = ones_resp

    return {
        "W1N_T": np.ascontiguousarray(W1N.T).astype(BF16),  # [126, 127]
        "WS_T": np.ascontiguousarray(WS.T).astype(BF16),    # [2, 127]
        "WA_T": np.ascontiguousarray(WA.T).astype(BF16),    # [125, 127]
        "WD_T": np.ascontiguousarray(WDm.T).astype(BF16),   # [126, 125]
        "BIAS": bias.astype(np.float32),                    # [127, 1]
        "S_INIT": np.zeros((1, N), BF16),                   # I_{-1} = 0
    }


def build_program(reps: int = 1):
    """Emit the single-core SPMD program. Returns (nc, weight_arrays)."""
    from concourse import bacc, mybir, tile

    dt = mybir.dt
    w = build_weights()
    wdtypes = {
        "W1N_T": dt.bfloat16, "WS_T": dt.bfloat16, "WA_T": dt.bfloat16,
        "WD_T": dt.bfloat16, "BIAS": dt.float32, "S_INIT": dt.bfloat16,
    }

    nc = bacc.Bacc("TRN2", target_bir_lowering=False, debug=False)

    X = nc.dram_tensor("X", [T, N], dt.bfloat16, kind="ExternalInput")
    Y = nc.dram_tensor("Y", [T, N], dt.bfloat16, kind="ExternalOutput")
    wd = {
        name: nc.dram_tensor(name, list(arr.shape), wdtypes[name], kind="ExternalInput")
        for name, arr in w.items()
    }

    with tile.TileContext(nc) as tc:
        with (
            tc.tile_pool(name="wpool", bufs=1) as wpool,
            tc.tile_pool(name="io", bufs=3) as io,
            tc.tile_pool(name="opool", bufs=6) as opool,
            tc.tile_pool(name="apool", bufs=2) as apool,
            tc.tile_pool(name="psO", bufs=2, space="PSUM") as psO,
            tc.tile_pool(name="psD", bufs=2, space="PSUM") as psD,
        ):
            # weights -> SBUF once
            wt = {}
            for name, arr in w.items():
                t_ = wpool.tile(list(arr.shape), wdtypes[name], tag=name)
                nc.sync.dma_start(out=t_[:], in_=wd[name][:])
                wt[name] = t_

            for rep in range(reps):
                # first chunk's rhs: I_{-1}=0 row from S_INIT, rest from X
                rhs = io.tile([126, N], dt.bfloat16, tag="rhs")
                nc.sync.dma_start(out=rhs[0:1, :], in_=wd["S_INIT"][:])
                nc.sync.dma_start(out=rhs[1:126, :], in_=X[0:L, :])

                prev_out = None
                for k in range(NCH):
                    if k + 1 < NCH:
                        rhs_next = io.tile([126, N], dt.bfloat16, tag="rhs")
                        nc.sync.dma_start(
                            out=rhs_next[:, :],
                            in_=X[(k + 1) * L - 1:(k + 2) * L, :],
                        )

                    # first difference via PE (compute engines can't read
                    # partition base 1), then abs*c3 on ACT per half
                    ps_d = [
                        psD.tile([L, NH], dt.float32, tag="D", name=f"psd{h}")
                        for h in range(2)
                    ]
                    for h in range(2):
                        for b_ in range(2):
                            c0 = b_ * NB
                            nc.tensor.matmul(
                                ps_d[h][:, c0:c0 + NB],
                                wt["WD_T"][:],
                                rhs[:, h * NH + c0:h * NH + c0 + NB],
                                start=True, stop=True,
                            )
                    a_t = apool.tile([L, N], dt.bfloat16, tag="a")
                    for h in range(2):
                        nc.scalar.activation(
                            a_t[:, h * NH:(h + 1) * NH], ps_d[h][:],
                            func=mybir.ActivationFunctionType.Abs,
                            scale=float(C3),
                        )

                    ps_o = [
                        psO.tile([127, NH], dt.float32, tag="O", name=f"pso{h}")
                        for h in range(2)
                    ]
                    for h in range(2):
                        for b_ in range(2):
                            c0 = b_ * NB
                            nc.tensor.matmul(
                                ps_o[h][:, c0:c0 + NB],
                                wt["W1N_T"][:],
                                rhs[:, h * NH + c0:h * NH + c0 + NB],
                                start=True, stop=False,
                            )
                    for h in range(2):
                        for b_ in range(2):
                            c0 = b_ * NB
                            nc.tensor.matmul(
                                ps_o[h][:, c0:c0 + NB],
                                wt["WA_T"][:],
                                a_t[:, h * NH + c0:h * NH + c0 + NB],
                                start=False, stop=(k == 0),
                            )
                    if k > 0:
                        # carry response: read carries straight from the
                        # previous chunk's out tiles (rows 0-1, SBUF bf16)
                        for h in range(2):
                            for b_ in range(2):
                                c0 = b_ * NB
                                nc.tensor.matmul(
                                    ps_o[h][:, c0:c0 + NB],
                                    wt["WS_T"][:],
                                    prev_out[h][0:2, c0:c0 + NB],
                                    start=False, stop=True,
                                )

                    # PSUM->SBUF bias-add copies, one half each on ACT / DVE
                    out_h = [
                        opool.tile([127, NH], dt.bfloat16, tag="out", name=f"out{h}")
                        for h in range(2)
                    ]
                    for h in range(2):
                        nc.vector.tensor_scalar(
                            out=out_h[h][:], in0=ps_o[h][:],
                            scalar1=wt["BIAS"][:], scalar2=None,
                            op0=mybir.AluOpType.add,
                        )

                    for h in range(2):
                        nc.sync.dma_start(
                            out=Y[k * L:(k + 1) * L, h * NH:(h + 1) * NH],
                            in_=out_h[h][2:127, :],
                        )
                    prev_out = out_h
                    rhs = rhs_next if k + 1 < NCH else None

    nc.compile()
    return nc, w


_PROGRAM_CACHE = {}


def _get_program():
    if "nc" not in _PROGRAM_CACHE:
        nc, w = build_program()
        _PROGRAM_CACHE["nc"] = nc
        _PROGRAM_CACHE["w"] = w
    return _PROGRAM_CACHE["nc"], _PROGRAM_CACHE["w"]


def kernel(I_in: np.ndarray) -> np.ndarray:
    """Full-input entry point: I_in [8, 2000, 2048] fp32 -> out same shape."""
    from concourse.bass_utils import run_bass_kernel_spmd

    nc, w = _get_program()
    I_bf = np.ascontiguousarray(I_in, dtype=np.float32).astype(BF16)
    in_maps = [
        {"X": I_bf[b], **{name: arr for name, arr in w.items()}}
        for b in range(B)
    ]
    last_err = None
    for _attempt in range(3):
        try:
            res = run_bass_kernel_spmd(nc, in_maps, list(range(B)))
            out = np.stack(
                [np.asarray(res.results[b]["Y"]).astype(np.float32) for b in range(B)],
                axis=0,
            )
            g = (MU * (1.0 - LAM_D ** (np.arange(T, dtype=np.float64) + 1))).astype(
                np.float32
            )
            return out + g[None, :, None]
        except Exception as e:  # transient device errors: retry
            last_err = e
            import time as _time
            _time.sleep(5)
    raise last_err


if __name__ == "__main__":
    rng = np.random.default_rng(0)
    I = rng.standard_normal((B, T, N), dtype=np.float32)
    out = kernel(I)
    print(out.shape, out.dtype, np.abs(out).max())
